# revision 30
# baseline (speedup 1.0000x reference)
# Distributed Trainium2 kernel for nn_Attn (general attention scores + softmax).
#
# reference:
#   proj   = einsum('tbh,dh->tbd', encoder_outputs, W)
#   scores = einsum('hb,tbh->bt', dec_hidden, proj)
#   out    = softmax(scores, axis=1)
#
# Algebraic rewrite: scores[b,t] = sum_h (W^T @ dec_hidden)[h,b] * enc[t,b,h].
# Precompute q = W^T @ dec_hidden on the PE (tiny), then one streaming pass
# over encoder_outputs with a fused multiply+row-reduce on VectorE -> purely
# HBM-bandwidth-bound.
#
# Sharding: T is split across the 8 cores (enc shard [T/8, B, H] per core);
# W and dec_hidden are replicated.  Local scores [128, 64] tiles are exchanged
# core-to-core with remote_dma_broadcast (SBUF->SBUF over the on-chip fabric,
# ~us) instead of an ncfw AllGather (~40us); a 1-byte collective issued at
# kernel start acts as the entry barrier and completes under the streaming.
# Softmax runs over a t-permuted layout (order-invariant) and the final
# normalize's read AP undoes the permutation.
import sys
from contextlib import ExitStack

for _p in ("/opt/trn_rl_repo", "/opt/pypackages"):
    if _p not in sys.path:
        sys.path.append(_p)

import numpy as np

import concourse.bass as bass
import concourse.bacc as bacc
import concourse.mybir as mybir
from concourse.bass_utils import run_bass_kernel_spmd

H = 1024
B = 16
T = 4096
NCORES = 8
T_L = T // NCORES          # 512 timesteps per core
ROWS = T_L * B             # 8192 (t,b) rows per core
NCHUNK = 16                # streaming chunks per core (2 MB each)
CROWS = ROWS // NCHUNK     # 512 rows per chunk
CSUB = CROWS // 128        # 4 [128, H] sub-tiles per chunk
NTILES = NCHUNK * CSUB     # 64
SUB = 8                    # partition-group factor (t_l % 8) used by the tail
RING = 8                   # 2MB ring slots over 4 physical buffers (buf3 = w_sb)
FP32 = mybir.dt.float32
BF16 = mybir.dt.bfloat16


def build_nc() -> bass.Bass:
    nc = bacc.Bacc(num_devices=NCORES)

    enc = nc.declare_dram_parameter("enc", [ROWS, H], FP32, isOutput=False)
    dec = nc.declare_dram_parameter("dec", [H, B], FP32, isOutput=False)
    w = nc.declare_dram_parameter("w", [H, H], FP32, isOutput=False)
    out = nc.declare_dram_parameter("out", [B, T_L], FP32, isOutput=True)
    sel = nc.declare_dram_parameter("sel", [128, B], FP32, isOutput=False)

    with ExitStack() as ctx:
        # w_sb doubles as ring slot 3 once the q matmuls are done with W
        w_sb = ctx.enter_context(nc.sbuf_tensor("w_sb", [128, SUB * H], FP32))
        ring0 = ctx.enter_context(nc.sbuf_tensor("ring0", [128, SUB * H], FP32))
        ring1 = ctx.enter_context(nc.sbuf_tensor("ring1", [128, SUB * H], FP32))
        ring2 = ctx.enter_context(nc.sbuf_tensor("ring2", [128, SUB * H], FP32))
        dec_sb = ctx.enter_context(nc.sbuf_tensor("dec_sb", [128, 8 * B], FP32))
        dec_rep = ctx.enter_context(nc.sbuf_tensor("dec_rep", [128, 8 * 128], FP32))
        q_tiled = ctx.enter_context(nc.sbuf_tensor("q_tiled", [128, H], FP32))
        scores_buf = ctx.enter_context(nc.sbuf_tensor("scores_buf", [128, NTILES], FP32))
        sel_sb = ctx.enter_context(nc.sbuf_tensor("sel_sb", [128, B], FP32))
        m_loc = ctx.enter_context(nc.sbuf_tensor("m_loc", [128, 1], FP32))
        m_row = ctx.enter_context(nc.sbuf_tensor("m_row", [1, 128], FP32))
        m_sc = ctx.enter_context(nc.sbuf_tensor("m_sc", [1, 1], FP32))
        m_bc = ctx.enter_context(nc.sbuf_tensor("m_bc", [128, 1], FP32))
        m_all = ctx.enter_context(nc.sbuf_tensor("m_all", [128, NCORES], FP32))
        negM = ctx.enter_context(nc.sbuf_tensor("negM", [128, 1], FP32))
        e_buf = ctx.enter_context(nc.sbuf_tensor("e_buf", [128, NTILES], FP32))
        sacc = ctx.enter_context(nc.sbuf_tensor("sacc", [128, 1], FP32))
        s128 = ctx.enter_context(nc.sbuf_tensor("s128", [128, 1], FP32))
        s_all = ctx.enter_context(nc.sbuf_tensor("s_all", [128, NCORES], FP32))
        s_glob = ctx.enter_context(nc.sbuf_tensor("s_glob", [128, 1], FP32))
        rinv16 = ctx.enter_context(nc.sbuf_tensor("rinv16", [B, 1], FP32))
        e3 = ctx.enter_context(nc.sbuf_tensor("e3", [B, SUB * NTILES], FP32))
        o2 = ctx.enter_context(nc.sbuf_tensor("o2", [B, T_L], FP32))
        psum0 = ctx.enter_context(nc.psum_tensor("psum0", [128, 512], FP32))
        psum1 = ctx.enter_context(nc.psum_tensor("psum1", [128, 512], FP32))
        psum_s = ctx.enter_context(nc.psum_tensor("psum_s", [B, 1], FP32))

        sem_dec = ctx.enter_context(nc.semaphore("sem_dec"))
        wsems = [ctx.enter_context(nc.semaphore(f"sem_w{i}")) for i in range(8)]
        slot_sems = [ctx.enter_context(nc.semaphore(f"sem_s{i}")) for i in range(RING)]
        prep_sem = ctx.enter_context(nc.semaphore("prep_sem"))
        gatherM = ctx.enter_context(nc.semaphore("gatherM"))
        gatherS = ctx.enter_context(nc.semaphore("gatherS"))
        lsem_rdma = ctx.enter_context(nc.semaphore("lsem_rdma"))
        soft_sems = [ctx.enter_context(nc.semaphore(f"sem_soft{i}")) for i in range(SUB)]
        sem_sel = ctx.enter_context(nc.semaphore("sem_sel"))
        sem_mrow = ctx.enter_context(nc.semaphore("sem_mrow"))
        sem_final = ctx.enter_context(nc.semaphore("sem_final"))
        v_prep = ctx.enter_context(nc.semaphore("v_prep"))
        pe_done = ctx.enter_context(nc.semaphore("pe_done"))
        v_done = ctx.enter_context(nc.semaphore("v_done"))
        v_m1 = ctx.enter_context(nc.semaphore("v_m1"))
        v_m2 = ctx.enter_context(nc.semaphore("v_m2"))
        g_mbc = ctx.enter_context(nc.semaphore("g_mbc"))
        v_m3 = ctx.enter_context(nc.semaphore("v_m3"))
        a_e = ctx.enter_context(nc.semaphore("a_e"))
        pe_s = ctx.enter_context(nc.semaphore("pe_s"))
        v_s = ctx.enter_context(nc.semaphore("v_s"))
        v_fin = ctx.enter_context(nc.semaphore("v_fin"))
        block = ctx.enter_context(nc.Block())

        rings = [ring0, ring1, ring2, w_sb]
        psums = [psum0, psum1]

        @block.sync
        def _(sync):
            # dec first (tiny), then W in 8 pipelined chunks, then enc chunks
            sync.dma_start(
                out=dec_sb[:],
                in_=dec[:].rearrange("(dc p) b -> p dc b", p=128),
            ).then_inc(sem_dec, 16)
            sync.dma_start(out=sel_sb[:], in_=sel[:]).then_inc(sem_sel, 16)
            def w_chunk(dc):
                sync.dma_start(
                    out=w_sb[:, dc * H:(dc + 1) * H],
                    in_=w[dc * 128:(dc + 1) * 128, :],
                ).then_inc(wsems[dc], 16)
            def enc_chunk(c):
                s = c % RING
                if c == 6:
                    # slots 6/7 live in w_sb: wait for q matmuls to release W
                    sync.wait_ge(pe_done, 1)
                if c >= RING:
                    sync.wait_ge(v_done, c - RING + 1)
                sync.dma_start(
                    out=rings[s // 2][:, (s % 2) * CSUB * H:(s % 2 + 1) * CSUB * H],
                    in_=enc[c * CROWS:(c + 1) * CROWS, :].rearrange(
                        "(j p) h -> p j h", p=128
                    ),
                ).then_inc(slot_sems[s], 16)

            for dc in range(8):
                w_chunk(dc)
            for c in range(NCHUNK):
                enc_chunk(c)

        @block.vector
        def _(vector):
            # dec_rep[p, (dc k b)] = dec_sb[p, (dc b)]  (repeat 8x along k)
            vector.wait_ge(sem_dec, 16)
            vector.tensor_copy(
                dec_rep[:].rearrange("p (dc k b) -> p dc k b", dc=8, k=8),
                dec_sb[:]
                .rearrange("p (dc b) -> p dc b", dc=8)
                .unsqueeze(2)
                .broadcast_to([128, 8, 8, B]),
            ).then_inc(v_prep, 1)

            # q_tiled[p, h] = q[h, p%16] from PSUM
            vector.wait_ge(pe_done, 1)
            vector.tensor_copy(q_tiled[:, 0:512], psum0[:])
            vector.tensor_copy(q_tiled[:, 512:1024], psum1[:])
            vector.drain()

            # main streaming loop: fused multiply + free-axis reduce, in-place
            for c in range(NCHUNK):
                s = c % RING
                vector.wait_ge(slot_sems[s], 16 * (c // RING + 1))
                for j in range(CSUB):
                    i = c * CSUB + j
                    off = (s % 2) * CSUB * H + j * H
                    ins = vector.scalar_tensor_tensor(
                        out=rings[s // 2][:, off:off + H],
                        in0=rings[s // 2][:, off:off + H],
                        scalar=0.0,
                        in1=q_tiled[:],
                        op0=mybir.AluOpType.add,
                        op1=mybir.AluOpType.mult,
                        accum_out=scores_buf[:, i:i + 1],
                    )
                    if j == CSUB - 1:
                        ins.then_inc(v_done, 1)

            # ---- stats-exchange softmax ----
            # local per-partition max over the 64 score columns
            vector.memset(s128[:], 0.0)
            vector.drain()
            vector.tensor_reduce(
                m_loc[:],
                scores_buf[:],
                axis=mybir.AxisListType.X,
                op=mybir.AluOpType.max,
            ).then_inc(v_m1, 1)
            # cross-partition: m_row arrives via SBUF->SBUF DMA; reduce to a
            # scalar, gpsimd broadcasts it back across partitions
            vector.wait_ge(sem_mrow, 16)
            vector.tensor_reduce(
                m_sc[:],
                m_row[:],
                axis=mybir.AxisListType.X,
                op=mybir.AluOpType.max,
            ).then_inc(v_m2, 1)
            # after the remote M exchange: global max -> -M
            vector.wait_ge(gatherM, 16)
            vector.tensor_reduce(
                negM[:],
                m_all[:],
                axis=mybir.AxisListType.X,
                op=mybir.AluOpType.max,
                negate=True,
            ).then_inc(v_m3, 1)
            # per-b global sums after the s exchange
            vector.wait_ge(pe_s, 1)
            vector.drain()
            vector.tensor_copy(s128[0:B, :], psum_s[:]).then_inc(v_s, 1)
            vector.wait_ge(gatherS, 16)
            vector.tensor_reduce(
                s_glob[:],
                s_all[:],
                axis=mybir.AxisListType.X,
                op=mybir.AluOpType.add,
            )
            vector.drain()
            vector.reciprocal(rinv16[:], s_glob[0:B, :])
            vector.drain()
            # final normalize on the local slice; e3 free layout (k, i),
            # t_l = i*8 + k
            for k in range(SUB):
                vector.wait_ge(soft_sems[k], 16)
            vector.tensor_scalar_mul(
                o2[:].rearrange("b (i k) -> b i k", k=SUB),
                e3[:].rearrange("b (k i) -> b i k", k=SUB),
                rinv16[:],
            ).then_inc(v_fin, 1)

        @block.tensor
        def _(tensor):
            # q_tiled[p, h] = sum_d dec[d, p%16] * W[d, h], chunk-pipelined on W
            tensor.wait_ge(v_prep, 1)
            last = None
            for dc in range(8):
                tensor.wait_ge(wsems[dc], 16)
                for half in range(2):
                    last = tensor.matmul(
                        psums[half][:],
                        dec_rep[:, dc * 128:(dc + 1) * 128],
                        w_sb[:, dc * H + half * 512: dc * H + half * 512 + 512],
                        start=(dc == 0),
                        stop=(dc == 7),
                    )
            last.then_inc(pe_done, 1)
            # per-b partial sums: psum_s[b] = sum_p sel[p, b] * sacc[p]
            tensor.wait_ge(sem_sel, 16)
            tensor.wait_ge(a_e, 1)
            tensor.matmul(psum_s[:], sel_sb[:], sacc[:]).then_inc(pe_s, 1)

        @block.gpsimd
        def _(gpsimd):
            # pre-generate the broadcast descriptors (hides SWDGE latency);
            # slot = my core id (the AP offset is runtime-computed)
            pid = gpsimd.partition_id()
            gpsimd.remote_dma_broadcast(
                out_ap=m_all[:, bass.ds(pid, 1)],
                in_ap=m_bc[:],
                remote_sem=gatherM,
                local_sem=lsem_rdma,
                rdests=[(0, k) for k in range(NCORES)],
            ).then_inc(prep_sem, 1)
            gpsimd.remote_dma_broadcast(
                out_ap=s_all[:, bass.ds(pid, 1)],
                in_ap=s128[:],
                remote_sem=gatherS,
                local_sem=lsem_rdma,
                rdests=[(0, k) for k in range(NCORES)],
            ).then_inc(prep_sem, 1)
            gpsimd.wait_ge(prep_sem, 2)
            # entry barrier (ncfw prelude AllGather, overlaps the stream phase):
            # remote SBUF writes are only safe once every peer started its NEFF
            gpsimd.bir_kernel_barrier_wait([list(range(NCORES))])
            # broadcast the local max scalar across partitions, then exchange
            gpsimd.wait_ge(v_m2, 1)
            gpsimd.partition_broadcast(m_bc[:], m_sc[:]).then_inc(g_mbc, 1)
            gpsimd.wait_ge(g_mbc, 1)
            gpsimd.trigger_dma()
            gpsimd.wait_ge(v_s, 1)
            gpsimd.trigger_dma()
            gpsimd.wait_ge(lsem_rdma, 32)  # sends complete

        @block.scalar
        def _(scalar):
            # local max cross-partition hop: [128,1] -> [1,128]
            scalar.wait_ge(v_m1, 1)
            scalar.dma_start(
                out=m_row[:],
                in_=m_loc[:],
            ).then_inc(sem_mrow, 16)
            # exp(x - M) on the local tile with fused per-partition sums
            scalar.wait_ge(v_m3, 1)
            scalar.activation(
                e_buf[:],
                scores_buf[:],
                mybir.ActivationFunctionType.Exp,
                bias=negM[:],
                scale=1.0,
                accum_out=sacc[:],
            ).then_inc(a_e, 1)
            # rearrange e_buf [p=(k b), i] -> e3 [b, (k i)]  (overlaps the s
            # exchange)
            scalar.wait_ge(a_e, 1)
            for k in range(SUB):
                scalar.dma_start(
                    out=e3[:, k * NTILES:(k + 1) * NTILES],
                    in_=e_buf[k * B:(k + 1) * B, :],
                ).then_inc(soft_sems[k], 16)
            # store this core's output slice
            scalar.wait_ge(v_fin, 1)
            scalar.dma_start(out=out[:], in_=o2[:]).then_inc(sem_final, 16)
            scalar.wait_ge(sem_final, 16)

    nc.compile()
    return nc


def make_in_maps(dec_hidden, encoder_outputs, W):
    dec_np = np.ascontiguousarray(np.asarray(dec_hidden, dtype=np.float32))
    enc_np = np.ascontiguousarray(np.asarray(encoder_outputs, dtype=np.float32))
    w_np = np.ascontiguousarray(np.asarray(W, dtype=np.float32))
    assert dec_np.shape == (H, B)
    assert enc_np.shape == (T, B, H)
    assert w_np.shape == (H, H)
    sel_np = np.zeros((128, B), dtype=np.float32)
    sel_np[np.arange(128), np.arange(128) % B] = 1.0
    in_maps = []
    for c in range(NCORES):
        shard = np.ascontiguousarray(
            enc_np[c * T_L:(c + 1) * T_L].reshape(ROWS, H)
        )
        in_maps.append({"enc": shard, "dec": dec_np, "w": w_np, "sel": sel_np})
    return in_maps


def _install_ntff_hook():
    """The image's antenv lacks axon_hooks; shim it and register the
    ctypes NTFF profile hook so trace=True works under axon."""
    import types

    if "antenv.axon_hooks" in sys.modules:
        return
    import antenv

    mod = types.ModuleType("antenv.axon_hooks")
    state = {"hook": None}
    mod.set_axon_ntff_profile_hook = lambda h: state.__setitem__("hook", h)
    mod.get_axon_ntff_profile_hook = lambda: state["hook"]
    sys.modules["antenv.axon_hooks"] = mod
    antenv.axon_hooks = mod
    try:
        from trn_agent_boot.trn_boot import _ntff_profile_via_ctypes

        mod.set_axon_ntff_profile_hook(
            _ntff_profile_via_ctypes("/opt/axon/libaxon_pjrt.so")
        )
    except Exception as e:  # degrade to no tracing
        print(f"ntff hook install failed: {e}", file=sys.stderr)


def run(dec_hidden, encoder_outputs, W, trace=False):
    if trace:
        _install_ntff_hook()
    nc = build_nc()
    in_maps = make_in_maps(dec_hidden, encoder_outputs, W)
    res = run_bass_kernel_spmd(
        nc, in_maps, core_ids=list(range(NCORES)), trace=trace
    )
    out = np.concatenate(
        [np.asarray(res.results[c]["out"], dtype=np.float32)
         for c in range(NCORES)],
        axis=1,
    )
    return out, res


def kernel(dec_hidden, encoder_outputs, W):
    out, _ = run(dec_hidden, encoder_outputs, W, trace=False)
    return out


# revision 31
# speedup vs baseline: 1.0960x; 1.0960x over previous
# Distributed Trainium2 kernel for nn_Attn (general attention scores + softmax).
#
# reference:
#   proj   = einsum('tbh,dh->tbd', encoder_outputs, W)
#   scores = einsum('hb,tbh->bt', dec_hidden, proj)
#   out    = softmax(scores, axis=1)
#
# Algebraic rewrite: scores[b,t] = sum_h (W^T @ dec_hidden)[h,b] * enc[t,b,h].
# Precompute q = W^T @ dec_hidden on the PE (tiny), then one streaming pass
# over encoder_outputs with a fused multiply+row-reduce on VectorE -> purely
# HBM-bandwidth-bound.
#
# Sharding: T is split across the 8 cores (enc shard [T/8, B, H] per core);
# W and dec_hidden are replicated.  Local scores [128, 64] tiles are exchanged
# core-to-core with remote_dma_broadcast (SBUF->SBUF over the on-chip fabric,
# ~us) instead of an ncfw AllGather (~40us); a 1-byte collective issued at
# kernel start acts as the entry barrier and completes under the streaming.
# Softmax runs over a t-permuted layout (order-invariant) and the final
# normalize's read AP undoes the permutation.
import sys
from contextlib import ExitStack

for _p in ("/opt/trn_rl_repo", "/opt/pypackages"):
    if _p not in sys.path:
        sys.path.append(_p)

import numpy as np

import concourse.bass as bass
import concourse.bacc as bacc
import concourse.mybir as mybir
from concourse import library_config
from concourse.bass_utils import run_bass_kernel_spmd

H = 1024
B = 16
T = 4096
NCORES = 8
T_L = T // NCORES          # 512 timesteps per core
ROWS = T_L * B             # 8192 (t,b) rows per core
NCHUNK = 16                # streaming chunks per core (2 MB each)
CROWS = ROWS // NCHUNK     # 512 rows per chunk
CSUB = CROWS // 128        # 4 [128, H] sub-tiles per chunk
NTILES = NCHUNK * CSUB     # 64
SUB = 8                    # partition-group factor (t_l % 8) used by the tail
RING = 8                   # 2MB ring slots over 4 physical buffers (buf3 = w_sb)
FP32 = mybir.dt.float32
BF16 = mybir.dt.bfloat16


def build_nc() -> bass.Bass:
    nc = bacc.Bacc(num_devices=NCORES)

    enc = nc.declare_dram_parameter("enc", [ROWS, H], FP32, isOutput=False)
    dec = nc.declare_dram_parameter("dec", [H, B], FP32, isOutput=False)
    w = nc.declare_dram_parameter("w", [H, H], FP32, isOutput=False)
    out = nc.declare_dram_parameter("out", [B, T_L], FP32, isOutput=True)
    sel = nc.declare_dram_parameter("sel", [128, B], FP32, isOutput=False)

    with ExitStack() as ctx:
        # w_sb doubles as ring slot 3 once the q matmuls are done with W
        w_sb = ctx.enter_context(nc.sbuf_tensor("w_sb", [128, SUB * H], FP32))
        ring0 = ctx.enter_context(nc.sbuf_tensor("ring0", [128, SUB * H], FP32))
        ring1 = ctx.enter_context(nc.sbuf_tensor("ring1", [128, SUB * H], FP32))
        ring2 = ctx.enter_context(nc.sbuf_tensor("ring2", [128, SUB * H], FP32))
        dec_sb = ctx.enter_context(nc.sbuf_tensor("dec_sb", [128, 8 * B], FP32))
        dec_rep = ctx.enter_context(nc.sbuf_tensor("dec_rep", [128, 8 * 128], FP32))
        q_tiled = ctx.enter_context(nc.sbuf_tensor("q_tiled", [128, H], FP32))
        scores_buf = ctx.enter_context(nc.sbuf_tensor("scores_buf", [128, NTILES], FP32))
        sel_sb = ctx.enter_context(nc.sbuf_tensor("sel_sb", [128, B], FP32))
        m_loc = ctx.enter_context(nc.sbuf_tensor("m_loc", [128, 1], FP32))
        m_row = ctx.enter_context(nc.sbuf_tensor("m_row", [1, 128], FP32))
        m_sc = ctx.enter_context(nc.sbuf_tensor("m_sc", [1, 1], FP32))
        m_bc = ctx.enter_context(nc.sbuf_tensor("m_bc", [128, 1], FP32))
        m_all = ctx.enter_context(nc.sbuf_tensor("m_all", [128, NCORES], FP32))
        negM = ctx.enter_context(nc.sbuf_tensor("negM", [128, 1], FP32))
        e_buf = ctx.enter_context(nc.sbuf_tensor("e_buf", [128, NTILES], FP32))
        sacc = ctx.enter_context(nc.sbuf_tensor("sacc", [128, 1], FP32))
        s128 = ctx.enter_context(nc.sbuf_tensor("s128", [128, 1], FP32))
        s_all = ctx.enter_context(nc.sbuf_tensor("s_all", [128, NCORES], FP32))
        s_glob = ctx.enter_context(nc.sbuf_tensor("s_glob", [128, 1], FP32))
        rinv16 = ctx.enter_context(nc.sbuf_tensor("rinv16", [B, 1], FP32))
        e3 = ctx.enter_context(nc.sbuf_tensor("e3", [B, SUB * NTILES], FP32))
        o2 = ctx.enter_context(nc.sbuf_tensor("o2", [B, T_L], FP32))
        psum0 = ctx.enter_context(nc.psum_tensor("psum0", [128, 512], FP32))
        psum1 = ctx.enter_context(nc.psum_tensor("psum1", [128, 512], FP32))
        psum_s = ctx.enter_context(nc.psum_tensor("psum_s", [B, 1], FP32))

        sem_dec = ctx.enter_context(nc.semaphore("sem_dec"))
        wsems = [ctx.enter_context(nc.semaphore(f"sem_w{i}")) for i in range(8)]
        slot_sems = [ctx.enter_context(nc.semaphore(f"sem_s{i}")) for i in range(RING)]
        prep_sem = ctx.enter_context(nc.semaphore("prep_sem"))
        gatherM = ctx.enter_context(nc.semaphore("gatherM"))
        gatherS = ctx.enter_context(nc.semaphore("gatherS"))
        lsem_rdma = ctx.enter_context(nc.semaphore("lsem_rdma"))
        soft_sems = [ctx.enter_context(nc.semaphore(f"sem_soft{i}")) for i in range(SUB)]
        sem_sel = ctx.enter_context(nc.semaphore("sem_sel"))
        sem_mrow = ctx.enter_context(nc.semaphore("sem_mrow"))
        sem_final = ctx.enter_context(nc.semaphore("sem_final"))
        v_prep = ctx.enter_context(nc.semaphore("v_prep"))
        pe_done = ctx.enter_context(nc.semaphore("pe_done"))
        v_done = ctx.enter_context(nc.semaphore("v_done"))
        v_m1 = ctx.enter_context(nc.semaphore("v_m1"))
        v_m2 = ctx.enter_context(nc.semaphore("v_m2"))
        g_mbc = ctx.enter_context(nc.semaphore("g_mbc"))
        v_m3 = ctx.enter_context(nc.semaphore("v_m3"))
        a_e = ctx.enter_context(nc.semaphore("a_e"))
        pe_s = ctx.enter_context(nc.semaphore("pe_s"))
        v_s = ctx.enter_context(nc.semaphore("v_s"))
        v_fin = ctx.enter_context(nc.semaphore("v_fin"))
        block = ctx.enter_context(nc.Block())

        rings = [ring0, ring1, ring2, w_sb]
        psums = [psum0, psum1]

        @block.sync
        def _(sync):
            # dec first (tiny), then W in 8 pipelined chunks, then enc chunks
            sync.dma_start(
                out=dec_sb[:],
                in_=dec[:].rearrange("(dc p) b -> p dc b", p=128),
            ).then_inc(sem_dec, 16)

            def w_chunk(dc):
                sync.dma_start(
                    out=w_sb[:, dc * H:(dc + 1) * H],
                    in_=w[dc * 128:(dc + 1) * 128, :],
                ).then_inc(wsems[dc], 16)
            def enc_chunk(c):
                s = c % RING
                if c == 6:
                    # slots 6/7 live in w_sb: wait for q matmuls to release W
                    sync.wait_ge(pe_done, 1)
                if c >= RING:
                    sync.wait_ge(v_done, c - RING + 1)
                sync.dma_start(
                    out=rings[s // 2][:, (s % 2) * CSUB * H:(s % 2 + 1) * CSUB * H],
                    in_=enc[c * CROWS:(c + 1) * CROWS, :].rearrange(
                        "(j p) h -> p j h", p=128
                    ),
                ).then_inc(slot_sems[s], 16)

            for dc in range(8):
                w_chunk(dc)
            sync.dma_start(out=sel_sb[:], in_=sel[:]).then_inc(sem_sel, 16)
            for c in range(NCHUNK):
                enc_chunk(c)
            # first half of the e_buf -> e3 rearrange (other half on scalar)
            sync.wait_ge(a_e, 1)
            for k in range(SUB // 2):
                sync.dma_start(
                    out=e3[:, k * NTILES:(k + 1) * NTILES],
                    in_=e_buf[k * B:(k + 1) * B, :],
                ).then_inc(soft_sems[k], 16)

        @block.vector
        def _(vector):
            # dec_rep[p, (dc k b)] = dec_sb[p, (dc b)]  (repeat 8x along k)
            vector.wait_ge(sem_dec, 16)
            vector.tensor_copy(
                dec_rep[:].rearrange("p (dc k b) -> p dc k b", dc=8, k=8),
                dec_sb[:]
                .rearrange("p (dc b) -> p dc b", dc=8)
                .unsqueeze(2)
                .broadcast_to([128, 8, 8, B]),
            ).then_inc(v_prep, 1)

            # q_tiled[p, h] = q[h, p%16] from PSUM
            vector.wait_ge(pe_done, 1)
            vector.tensor_copy(q_tiled[:, 0:512], psum0[:])
            vector.tensor_copy(q_tiled[:, 512:1024], psum1[:])
            vector.drain()

            # main streaming loop: fused multiply + free-axis reduce, in-place
            for c in range(NCHUNK):
                s = c % RING
                vector.wait_ge(slot_sems[s], 16 * (c // RING + 1))
                for j in range(CSUB):
                    i = c * CSUB + j
                    off = (s % 2) * CSUB * H + j * H
                    ins = vector.scalar_tensor_tensor(
                        out=rings[s // 2][:, off:off + H],
                        in0=rings[s // 2][:, off:off + H],
                        scalar=0.0,
                        in1=q_tiled[:],
                        op0=mybir.AluOpType.add,
                        op1=mybir.AluOpType.mult,
                        accum_out=scores_buf[:, i:i + 1],
                    )
                    if j == CSUB - 1:
                        ins.then_inc(v_done, 1)

            # ---- stats-exchange softmax ----
            # local per-partition max over the 64 score columns
            vector.memset(s128[:], 0.0)
            vector.drain()
            vector.tensor_reduce(
                m_loc[:],
                scores_buf[:],
                axis=mybir.AxisListType.X,
                op=mybir.AluOpType.max,
            ).then_inc(v_m1, 1)
            # cross-partition: m_row arrives via SBUF->SBUF DMA; reduce to a
            # scalar, gpsimd broadcasts it back across partitions
            vector.wait_ge(sem_mrow, 16)
            vector.tensor_reduce(
                m_sc[:],
                m_row[:],
                axis=mybir.AxisListType.X,
                op=mybir.AluOpType.max,
            ).then_inc(v_m2, 1)
            # after the remote M exchange: global max -> -M
            vector.wait_ge(gatherM, 16)
            vector.tensor_reduce(
                negM[:],
                m_all[:],
                axis=mybir.AxisListType.X,
                op=mybir.AluOpType.max,
                negate=True,
            ).then_inc(v_m3, 1)
            # per-b global sums after the s exchange
            vector.wait_ge(pe_s, 1)
            vector.drain()
            vector.tensor_copy(s128[0:B, :], psum_s[:]).then_inc(v_s, 1)
            vector.wait_ge(gatherS, 16)
            vector.tensor_reduce(
                s_glob[:],
                s_all[:],
                axis=mybir.AxisListType.X,
                op=mybir.AluOpType.add,
            )
            vector.drain()
            vector.reciprocal(rinv16[:], s_glob[0:B, :])
            vector.drain()
            # final normalize on the local slice; e3 free layout (k, i),
            # t_l = i*8 + k
            for k in range(SUB):
                vector.wait_ge(soft_sems[k], 16)
            vector.tensor_scalar_mul(
                o2[:].rearrange("b (i k) -> b i k", k=SUB),
                e3[:].rearrange("b (k i) -> b i k", k=SUB),
                rinv16[:],
            ).then_inc(v_fin, 1)

        @block.tensor
        def _(tensor):
            # q_tiled[p, h] = sum_d dec[d, p%16] * W[d, h], chunk-pipelined on W
            tensor.wait_ge(v_prep, 1)
            last = None
            for dc in range(8):
                tensor.wait_ge(wsems[dc], 16)
                for half in range(2):
                    last = tensor.matmul(
                        psums[half][:],
                        dec_rep[:, dc * 128:(dc + 1) * 128],
                        w_sb[:, dc * H + half * 512: dc * H + half * 512 + 512],
                        start=(dc == 0),
                        stop=(dc == 7),
                    )
            last.then_inc(pe_done, 1)
            # per-b partial sums: psum_s[b] = sum_p sel[p, b] * sacc[p]
            tensor.wait_ge(sem_sel, 16)
            tensor.wait_ge(a_e, 1)
            tensor.matmul(psum_s[:], sel_sb[:], sacc[:]).then_inc(pe_s, 1)

        @block.gpsimd
        def _(gpsimd):
            # pre-generate the broadcast descriptors (hides SWDGE latency);
            # slot = my core id (the AP offset is runtime-computed)
            gpsimd.load_library(library_config.proxy)
            pid = gpsimd.partition_id()
            gpsimd.remote_dma_broadcast(
                out_ap=m_all[:, bass.ds(pid, 1)],
                in_ap=m_bc[:],
                remote_sem=gatherM,
                local_sem=lsem_rdma,
                rdests=[(0, k) for k in range(NCORES)],
            ).then_inc(prep_sem, 1)
            gpsimd.remote_dma_broadcast(
                out_ap=s_all[:, bass.ds(pid, 1)],
                in_ap=s128[:],
                remote_sem=gatherS,
                local_sem=lsem_rdma,
                rdests=[(0, k) for k in range(NCORES)],
            ).then_inc(prep_sem, 1)
            gpsimd.wait_ge(prep_sem, 2)
            # entry barrier (ncfw prelude AllGather, overlaps the stream phase):
            # remote SBUF writes are only safe once every peer started its NEFF
            gpsimd.bir_kernel_barrier_wait([list(range(NCORES))])
            # broadcast the local max scalar across partitions, then exchange
            gpsimd.wait_ge(v_m2, 1)
            gpsimd.partition_broadcast(m_bc[:], m_sc[:]).then_inc(g_mbc, 1)
            gpsimd.wait_ge(g_mbc, 1)
            gpsimd.trigger_dma()
            gpsimd.wait_ge(v_s, 1)
            gpsimd.trigger_dma()
            gpsimd.wait_ge(lsem_rdma, 32)  # sends complete

        @block.scalar
        def _(scalar):
            # local max cross-partition hop: [128,1] -> [1,128]
            scalar.wait_ge(v_m1, 1)
            scalar.dma_start(
                out=m_row[:],
                in_=m_loc[:],
            ).then_inc(sem_mrow, 16)
            # exp(x - M) on the local tile with fused per-partition sums
            scalar.wait_ge(v_m3, 1)
            scalar.activation(
                e_buf[:],
                scores_buf[:],
                mybir.ActivationFunctionType.Exp,
                bias=negM[:],
                scale=1.0,
                accum_out=sacc[:],
            ).then_inc(a_e, 1)
            # rearrange e_buf [p=(k b), i] -> e3 [b, (k i)]  (overlaps the s
            # exchange)
            scalar.wait_ge(a_e, 1)
            for k in range(SUB // 2, SUB):
                scalar.dma_start(
                    out=e3[:, k * NTILES:(k + 1) * NTILES],
                    in_=e_buf[k * B:(k + 1) * B, :],
                ).then_inc(soft_sems[k], 16)
            # store this core's output slice
            scalar.wait_ge(v_fin, 1)
            scalar.dma_start(out=out[:], in_=o2[:]).then_inc(sem_final, 16)
            scalar.wait_ge(sem_final, 16)

    nc.compile()
    return nc


def make_in_maps(dec_hidden, encoder_outputs, W):
    dec_np = np.ascontiguousarray(np.asarray(dec_hidden, dtype=np.float32))
    enc_np = np.ascontiguousarray(np.asarray(encoder_outputs, dtype=np.float32))
    w_np = np.ascontiguousarray(np.asarray(W, dtype=np.float32))
    assert dec_np.shape == (H, B)
    assert enc_np.shape == (T, B, H)
    assert w_np.shape == (H, H)
    sel_np = np.zeros((128, B), dtype=np.float32)
    sel_np[np.arange(128), np.arange(128) % B] = 1.0
    in_maps = []
    for c in range(NCORES):
        shard = np.ascontiguousarray(
            enc_np[c * T_L:(c + 1) * T_L].reshape(ROWS, H)
        )
        in_maps.append({"enc": shard, "dec": dec_np, "w": w_np, "sel": sel_np})
    return in_maps


def _install_ntff_hook():
    """The image's antenv lacks axon_hooks; shim it and register the
    ctypes NTFF profile hook so trace=True works under axon."""
    import types

    if "antenv.axon_hooks" in sys.modules:
        return
    import antenv

    mod = types.ModuleType("antenv.axon_hooks")
    state = {"hook": None}
    mod.set_axon_ntff_profile_hook = lambda h: state.__setitem__("hook", h)
    mod.get_axon_ntff_profile_hook = lambda: state["hook"]
    sys.modules["antenv.axon_hooks"] = mod
    antenv.axon_hooks = mod
    try:
        from trn_agent_boot.trn_boot import _ntff_profile_via_ctypes

        mod.set_axon_ntff_profile_hook(
            _ntff_profile_via_ctypes("/opt/axon/libaxon_pjrt.so")
        )
    except Exception as e:  # degrade to no tracing
        print(f"ntff hook install failed: {e}", file=sys.stderr)


def run(dec_hidden, encoder_outputs, W, trace=False):
    if trace:
        _install_ntff_hook()
    nc = build_nc()
    in_maps = make_in_maps(dec_hidden, encoder_outputs, W)
    res = run_bass_kernel_spmd(
        nc, in_maps, core_ids=list(range(NCORES)), trace=trace
    )
    out = np.concatenate(
        [np.asarray(res.results[c]["out"], dtype=np.float32)
         for c in range(NCORES)],
        axis=1,
    )
    return out, res


def kernel(dec_hidden, encoder_outputs, W):
    out, _ = run(dec_hidden, encoder_outputs, W, trace=False)
    return out


# revision 32
# speedup vs baseline: 1.1529x; 1.0519x over previous
# Distributed Trainium2 kernel for nn_Attn (general attention scores + softmax).
#
# reference:
#   proj   = einsum('tbh,dh->tbd', encoder_outputs, W)
#   scores = einsum('hb,tbh->bt', dec_hidden, proj)
#   out    = softmax(scores, axis=1)
#
# Algebraic rewrite: scores[b,t] = sum_h (W^T @ dec_hidden)[h,b] * enc[t,b,h].
# Precompute q = W^T @ dec_hidden on the PE (tiny), then one streaming pass
# over encoder_outputs with a fused multiply+row-reduce on VectorE -> purely
# HBM-bandwidth-bound.
#
# Sharding: T is split across the 8 cores (enc shard [T/8, B, H] per core);
# W and dec_hidden are replicated.  Local scores [128, 64] tiles are exchanged
# core-to-core with remote_dma_broadcast (SBUF->SBUF over the on-chip fabric,
# ~us) instead of an ncfw AllGather (~40us); a 1-byte collective issued at
# kernel start acts as the entry barrier and completes under the streaming.
# Softmax runs over a t-permuted layout (order-invariant) and the final
# normalize's read AP undoes the permutation.
import sys
from contextlib import ExitStack

for _p in ("/opt/trn_rl_repo", "/opt/pypackages"):
    if _p not in sys.path:
        sys.path.append(_p)

import numpy as np

import concourse.bass as bass
import concourse.bacc as bacc
import concourse.mybir as mybir
from concourse import library_config
from concourse.bass_utils import run_bass_kernel_spmd

H = 1024
B = 16
T = 4096
NCORES = 8
T_L = T // NCORES          # 512 timesteps per core
ROWS = T_L * B             # 8192 (t,b) rows per core
import os
NCHUNK = int(os.environ.get("K_NCHUNK", "16"))  # streaming chunks per core
CROWS = ROWS // NCHUNK     # rows per chunk
CSUB = CROWS // 128        # [128, H] sub-tiles per chunk
NTILES = NCHUNK * CSUB     # 64
SUB = 8                    # partition-group factor (t_l % 8) used by the tail
NBUF = 4                   # physical 4MB ring buffers (buf3 = w_sb)
RING = 32 // NCHUNK * NBUF // (32 // NCHUNK)  # placeholder, fixed below
RING = NCHUNK // 2 if NCHUNK >= 8 else NCHUNK  # slots
PER_BUF = RING // NBUF     # slots per physical buffer
FP32 = mybir.dt.float32
BF16 = mybir.dt.bfloat16


def build_nc() -> bass.Bass:
    nc = bacc.Bacc(num_devices=NCORES)

    enc = nc.declare_dram_parameter("enc", [ROWS, H], FP32, isOutput=False)
    dec = nc.declare_dram_parameter("dec", [H, B], FP32, isOutput=False)
    w = nc.declare_dram_parameter("w", [H, H], FP32, isOutput=False)
    out = nc.declare_dram_parameter("out", [B, T_L], FP32, isOutput=True)

    with ExitStack() as ctx:
        # w_sb doubles as ring slot 3 once the q matmuls are done with W
        w_sb = ctx.enter_context(nc.sbuf_tensor("w_sb", [128, SUB * H], FP32))
        ring0 = ctx.enter_context(nc.sbuf_tensor("ring0", [128, SUB * H], FP32))
        ring1 = ctx.enter_context(nc.sbuf_tensor("ring1", [128, SUB * H], FP32))
        ring2 = ctx.enter_context(nc.sbuf_tensor("ring2", [128, SUB * H], FP32))
        dec_sb = ctx.enter_context(nc.sbuf_tensor("dec_sb", [128, 8 * B], FP32))
        dec_rep = ctx.enter_context(nc.sbuf_tensor("dec_rep", [128, 8 * 128], FP32))
        q_tiled = ctx.enter_context(nc.sbuf_tensor("q_tiled", [128, H], FP32))
        scores_buf = ctx.enter_context(nc.sbuf_tensor("scores_buf", [128, NTILES], FP32))
        m_loc = ctx.enter_context(nc.sbuf_tensor("m_loc", [128, 1], FP32))
        m_row = ctx.enter_context(nc.sbuf_tensor("m_row", [1, 128], FP32))
        m_sc = ctx.enter_context(nc.sbuf_tensor("m_sc", [1, 1], FP32))
        m_bc = ctx.enter_context(nc.sbuf_tensor("m_bc", [128, 1], FP32))
        m_all = ctx.enter_context(nc.sbuf_tensor("m_all", [128, NCORES], FP32))
        negM = ctx.enter_context(nc.sbuf_tensor("negM", [128, 1], FP32))
        s3 = ctx.enter_context(nc.sbuf_tensor("s3", [B, SUB * NTILES], FP32))
        s128 = ctx.enter_context(nc.sbuf_tensor("s128", [128, 1], FP32))
        s_all = ctx.enter_context(nc.sbuf_tensor("s_all", [128, NCORES], FP32))
        s_glob = ctx.enter_context(nc.sbuf_tensor("s_glob", [128, 1], FP32))
        rinv16 = ctx.enter_context(nc.sbuf_tensor("rinv16", [B, 1], FP32))
        e3 = ctx.enter_context(nc.sbuf_tensor("e3", [B, SUB * NTILES], FP32))
        o2 = ctx.enter_context(nc.sbuf_tensor("o2", [B, T_L], FP32))
        psum0 = ctx.enter_context(nc.psum_tensor("psum0", [128, 512], FP32))
        psum1 = ctx.enter_context(nc.psum_tensor("psum1", [128, 512], FP32))

        sem_dec = ctx.enter_context(nc.semaphore("sem_dec"))
        wsems = [ctx.enter_context(nc.semaphore(f"sem_w{i}")) for i in range(8)]
        slot_sems = [ctx.enter_context(nc.semaphore(f"sem_s{i}")) for i in range(RING)]
        prep_sem = ctx.enter_context(nc.semaphore("prep_sem"))
        gatherM = ctx.enter_context(nc.semaphore("gatherM"))
        gatherS = ctx.enter_context(nc.semaphore("gatherS"))
        lsem_rdma = ctx.enter_context(nc.semaphore("lsem_rdma"))
        soft_sems = [ctx.enter_context(nc.semaphore(f"sem_soft{i}")) for i in range(SUB)]
        sem_mrow = ctx.enter_context(nc.semaphore("sem_mrow"))
        sem_final = ctx.enter_context(nc.semaphore("sem_final"))
        v_prep = ctx.enter_context(nc.semaphore("v_prep"))
        pe_done = ctx.enter_context(nc.semaphore("pe_done"))
        v_done = ctx.enter_context(nc.semaphore("v_done"))
        v_m1 = ctx.enter_context(nc.semaphore("v_m1"))
        v_m2 = ctx.enter_context(nc.semaphore("v_m2"))
        g_mbc = ctx.enter_context(nc.semaphore("g_mbc"))
        v_m3 = ctx.enter_context(nc.semaphore("v_m3"))
        a_e = ctx.enter_context(nc.semaphore("a_e"))
        v_s = ctx.enter_context(nc.semaphore("v_s"))
        v_fin = ctx.enter_context(nc.semaphore("v_fin"))
        block = ctx.enter_context(nc.Block())

        rings = [ring0, ring1, ring2, w_sb]
        psums = [psum0, psum1]

        @block.sync
        def _(sync):
            # dec first (tiny), then W in 8 pipelined chunks, then enc chunks
            sync.dma_start(
                out=dec_sb[:],
                in_=dec[:].rearrange("(dc p) b -> p dc b", p=128),
            ).then_inc(sem_dec, 16)

            def w_chunk(dc):
                sync.dma_start(
                    out=w_sb[:, dc * H:(dc + 1) * H],
                    in_=w[dc * 128:(dc + 1) * 128, :],
                ).then_inc(wsems[dc], 16)
            def enc_chunk(c):
                s = c % RING
                if s == (NBUF - 1) * PER_BUF and c == s:
                    # first slot living in w_sb: wait for q matmuls to
                    # release W
                    sync.wait_ge(pe_done, 1)
                if c >= RING:
                    sync.wait_ge(v_done, c - RING + 1)
                off = (s % PER_BUF) * CSUB * H
                sync.dma_start(
                    out=rings[s // PER_BUF][:, off:off + CSUB * H],
                    in_=enc[c * CROWS:(c + 1) * CROWS, :].rearrange(
                        "(j p) h -> p j h", p=128
                    ),
                ).then_inc(slot_sems[s], 16)

            for dc in range(8):
                w_chunk(dc)
            for c in range(NCHUNK):
                enc_chunk(c)
            # first half of the scores_buf -> s3 rearrange (rest on scalar)
            sync.wait_ge(v_done, NCHUNK)
            for k in range(SUB // 2):
                sync.dma_start(
                    out=s3[:, k * NTILES:(k + 1) * NTILES],
                    in_=scores_buf[k * B:(k + 1) * B, :],
                ).then_inc(soft_sems[k], 16)

        @block.vector
        def _(vector):
            # dec_rep[p, (dc k b)] = dec_sb[p, (dc b)]  (repeat 8x along k)
            vector.wait_ge(sem_dec, 16)
            vector.tensor_copy(
                dec_rep[:].rearrange("p (dc k b) -> p dc k b", dc=8, k=8),
                dec_sb[:]
                .rearrange("p (dc b) -> p dc b", dc=8)
                .unsqueeze(2)
                .broadcast_to([128, 8, 8, B]),
            ).then_inc(v_prep, 1)

            # q_tiled[p, h] = q[h, p%16] from PSUM
            vector.wait_ge(pe_done, 1)
            vector.tensor_copy(q_tiled[:, 0:512], psum0[:])
            vector.tensor_copy(q_tiled[:, 512:1024], psum1[:])
            vector.drain()

            # main streaming loop: fused multiply + free-axis reduce, in-place
            for c in range(NCHUNK):
                s = c % RING
                vector.wait_ge(slot_sems[s], 16 * (c // RING + 1))
                for j in range(CSUB):
                    i = c * CSUB + j
                    off = (s % PER_BUF) * CSUB * H + j * H
                    ins = vector.scalar_tensor_tensor(
                        out=rings[s // PER_BUF][:, off:off + H],
                        in0=rings[s // PER_BUF][:, off:off + H],
                        scalar=0.0,
                        in1=q_tiled[:],
                        op0=mybir.AluOpType.add,
                        op1=mybir.AluOpType.mult,
                        accum_out=scores_buf[:, i:i + 1],
                    )
                    if j == CSUB - 1:
                        ins.then_inc(v_done, 1)

            # ---- stats-exchange softmax ----
            # local per-partition max over the 64 score columns
            vector.memset(s128[:], 0.0)
            vector.drain()
            vector.tensor_reduce(
                m_loc[:],
                scores_buf[:],
                axis=mybir.AxisListType.X,
                op=mybir.AluOpType.max,
            ).then_inc(v_m1, 1)
            # cross-partition: m_row arrives via SBUF->SBUF DMA; reduce to a
            # scalar, gpsimd broadcasts it back across partitions
            vector.wait_ge(sem_mrow, 16)
            vector.tensor_reduce(
                m_sc[:],
                m_row[:],
                axis=mybir.AxisListType.X,
                op=mybir.AluOpType.max,
            ).then_inc(v_m2, 1)
            # after the remote M exchange: global max -> -M
            vector.wait_ge(gatherM, 16)
            vector.tensor_reduce(
                negM[:],
                m_all[:],
                axis=mybir.AxisListType.X,
                op=mybir.AluOpType.max,
                negate=True,
            ).then_inc(v_m3, 1)
            vector.wait_ge(gatherS, 16)
            vector.tensor_reduce(
                s_glob[:],
                s_all[:],
                axis=mybir.AxisListType.X,
                op=mybir.AluOpType.add,
            )
            vector.drain()
            vector.reciprocal(rinv16[:], s_glob[0:B, :])
            vector.drain()
            # final normalize on the local slice; e3 free layout (k, i),
            # t_l = i*8 + k
            vector.wait_ge(a_e, 1)
            vector.tensor_scalar_mul(
                o2[:].rearrange("b (i k) -> b i k", k=SUB),
                e3[:].rearrange("b (k i) -> b i k", k=SUB),
                rinv16[:],
            ).then_inc(v_fin, 1)

        @block.tensor
        def _(tensor):
            # q_tiled[p, h] = sum_d dec[d, p%16] * W[d, h], chunk-pipelined on W
            tensor.wait_ge(v_prep, 1)
            last = None
            for dc in range(8):
                tensor.wait_ge(wsems[dc], 16)
                for half in range(2):
                    last = tensor.matmul(
                        psums[half][:],
                        dec_rep[:, dc * 128:(dc + 1) * 128],
                        w_sb[:, dc * H + half * 512: dc * H + half * 512 + 512],
                        start=(dc == 0),
                        stop=(dc == 7),
                    )
            last.then_inc(pe_done, 1)

        @block.gpsimd
        def _(gpsimd):
            # pre-generate the broadcast descriptors (hides SWDGE latency);
            # slot = my core id (the AP offset is runtime-computed)
            gpsimd.load_library(library_config.proxy)
            pid = gpsimd.partition_id()
            gpsimd.remote_dma_broadcast(
                out_ap=m_all[:, bass.ds(pid, 1)],
                in_ap=m_bc[:],
                remote_sem=gatherM,
                local_sem=lsem_rdma,
                rdests=[(0, k) for k in range(NCORES)],
            ).then_inc(prep_sem, 1)
            gpsimd.remote_dma_broadcast(
                out_ap=s_all[:, bass.ds(pid, 1)],
                in_ap=s128[:],
                remote_sem=gatherS,
                local_sem=lsem_rdma,
                rdests=[(0, k) for k in range(NCORES)],
            ).then_inc(prep_sem, 1)
            gpsimd.wait_ge(prep_sem, 2)
            # entry barrier (ncfw prelude AllGather, overlaps the stream phase):
            # remote SBUF writes are only safe once every peer started its NEFF
            gpsimd.bir_kernel_barrier_wait([list(range(NCORES))])
            # broadcast the local max scalar across partitions, then exchange
            gpsimd.wait_ge(v_m2, 1)
            gpsimd.partition_broadcast(m_bc[:], m_sc[:]).then_inc(g_mbc, 1)
            gpsimd.wait_ge(g_mbc, 1)
            gpsimd.trigger_dma()
            gpsimd.wait_ge(a_e, 1)
            gpsimd.trigger_dma()
            gpsimd.wait_ge(lsem_rdma, 32)  # sends complete

        @block.scalar
        def _(scalar):
            # local max cross-partition hop: [128,1] -> [1,128]
            scalar.wait_ge(v_m1, 1)
            scalar.dma_start(
                out=m_row[:],
                in_=m_loc[:],
            ).then_inc(sem_mrow, 16)
            # second half of the raw-score rearrange
            scalar.wait_ge(v_done, NCHUNK)
            for k in range(SUB // 2, SUB):
                scalar.dma_start(
                    out=s3[:, k * NTILES:(k + 1) * NTILES],
                    in_=scores_buf[k * B:(k + 1) * B, :],
                ).then_inc(soft_sems[k], 16)
            # exp(x - M) in per-b layout; row sums land directly in s128
            for k in range(SUB):
                scalar.wait_ge(soft_sems[k], 16)
            scalar.wait_ge(v_m3, 1)
            scalar.activation(
                e3[:],
                s3[:],
                mybir.ActivationFunctionType.Exp,
                bias=negM[0:B, :],
                scale=1.0,
                accum_out=s128[0:B, :],
            ).then_inc(a_e, 1)
            # store this core's output slice
            scalar.wait_ge(v_fin, 1)
            scalar.dma_start(out=out[:], in_=o2[:]).then_inc(sem_final, 16)
            scalar.wait_ge(sem_final, 16)

    nc.compile()
    return nc


def make_in_maps(dec_hidden, encoder_outputs, W):
    dec_np = np.ascontiguousarray(np.asarray(dec_hidden, dtype=np.float32))
    enc_np = np.ascontiguousarray(np.asarray(encoder_outputs, dtype=np.float32))
    w_np = np.ascontiguousarray(np.asarray(W, dtype=np.float32))
    assert dec_np.shape == (H, B)
    assert enc_np.shape == (T, B, H)
    assert w_np.shape == (H, H)
    in_maps = []
    for c in range(NCORES):
        shard = np.ascontiguousarray(
            enc_np[c * T_L:(c + 1) * T_L].reshape(ROWS, H)
        )
        in_maps.append({"enc": shard, "dec": dec_np, "w": w_np})
    return in_maps


def _install_ntff_hook():
    """The image's antenv lacks axon_hooks; shim it and register the
    ctypes NTFF profile hook so trace=True works under axon."""
    import types

    if "antenv.axon_hooks" in sys.modules:
        return
    import antenv

    mod = types.ModuleType("antenv.axon_hooks")
    state = {"hook": None}
    mod.set_axon_ntff_profile_hook = lambda h: state.__setitem__("hook", h)
    mod.get_axon_ntff_profile_hook = lambda: state["hook"]
    sys.modules["antenv.axon_hooks"] = mod
    antenv.axon_hooks = mod
    try:
        from trn_agent_boot.trn_boot import _ntff_profile_via_ctypes

        mod.set_axon_ntff_profile_hook(
            _ntff_profile_via_ctypes("/opt/axon/libaxon_pjrt.so")
        )
    except Exception as e:  # degrade to no tracing
        print(f"ntff hook install failed: {e}", file=sys.stderr)


def run(dec_hidden, encoder_outputs, W, trace=False):
    if trace:
        _install_ntff_hook()
    nc = build_nc()
    in_maps = make_in_maps(dec_hidden, encoder_outputs, W)
    res = run_bass_kernel_spmd(
        nc, in_maps, core_ids=list(range(NCORES)), trace=trace
    )
    out = np.concatenate(
        [np.asarray(res.results[c]["out"], dtype=np.float32)
         for c in range(NCORES)],
        axis=1,
    )
    return out, res


def kernel(dec_hidden, encoder_outputs, W):
    out, _ = run(dec_hidden, encoder_outputs, W, trace=False)
    return out


# revision 33
# speedup vs baseline: 1.2201x; 1.0584x over previous
# Distributed Trainium2 kernel for nn_Attn (general attention scores + softmax).
#
# reference:
#   proj   = einsum('tbh,dh->tbd', encoder_outputs, W)
#   scores = einsum('hb,tbh->bt', dec_hidden, proj)
#   out    = softmax(scores, axis=1)
#
# Algebraic rewrite: scores[b,t] = sum_h (W^T @ dec_hidden)[h,b] * enc[t,b,h].
# Precompute q = W^T @ dec_hidden on the PE (tiny), then one streaming pass
# over encoder_outputs with a fused multiply+row-reduce on VectorE -> purely
# HBM-bandwidth-bound.
#
# Sharding: T is split across the 8 cores (enc shard [T/8, B, H] per core);
# W and dec_hidden are replicated.  Local scores [128, 64] tiles are exchanged
# core-to-core with remote_dma_broadcast (SBUF->SBUF over the on-chip fabric,
# ~us) instead of an ncfw AllGather (~40us); a 1-byte collective issued at
# kernel start acts as the entry barrier and completes under the streaming.
# Softmax runs over a t-permuted layout (order-invariant) and the final
# normalize's read AP undoes the permutation.
import sys
from contextlib import ExitStack

for _p in ("/opt/trn_rl_repo", "/opt/pypackages"):
    if _p not in sys.path:
        sys.path.append(_p)

import numpy as np

import concourse.bass as bass
import concourse.bacc as bacc
import concourse.mybir as mybir
from concourse import library_config
from concourse.bass_utils import run_bass_kernel_spmd

H = 1024
B = 16
T = 4096
NCORES = 8
T_L = T // NCORES          # 512 timesteps per core
ROWS = T_L * B             # 8192 (t,b) rows per core
import os
NCHUNK = int(os.environ.get("K_NCHUNK", "16"))  # streaming chunks per core
CROWS = ROWS // NCHUNK     # rows per chunk
CSUB = CROWS // 128        # [128, H] sub-tiles per chunk
NTILES = NCHUNK * CSUB     # 64
SUB = 8                    # partition-group factor (t_l % 8) used by the tail
NBUF = 4                   # physical 4MB ring buffers (buf3 = w_sb)
RING = 32 // NCHUNK * NBUF // (32 // NCHUNK)  # placeholder, fixed below
RING = NCHUNK // 2 if NCHUNK >= 8 else NCHUNK  # slots
PER_BUF = RING // NBUF     # slots per physical buffer
FP32 = mybir.dt.float32
BF16 = mybir.dt.bfloat16


def build_nc() -> bass.Bass:
    nc = bacc.Bacc(num_devices=NCORES)

    enc = nc.declare_dram_parameter("enc", [ROWS, H], FP32, isOutput=False)
    dec = nc.declare_dram_parameter("dec", [H, B], FP32, isOutput=False)
    w = nc.declare_dram_parameter("w", [H, H], FP32, isOutput=False)
    out = nc.declare_dram_parameter("out", [B, T_L], FP32, isOutput=True)

    with ExitStack() as ctx:
        # w_sb doubles as ring slot 3 once the q matmuls are done with W
        w_sb = ctx.enter_context(nc.sbuf_tensor("w_sb", [128, SUB * H], FP32))
        ring0 = ctx.enter_context(nc.sbuf_tensor("ring0", [128, SUB * H], FP32))
        ring1 = ctx.enter_context(nc.sbuf_tensor("ring1", [128, SUB * H], FP32))
        ring2 = ctx.enter_context(nc.sbuf_tensor("ring2", [128, SUB * H], FP32))
        dec_sb = ctx.enter_context(nc.sbuf_tensor("dec_sb", [128, 8 * B], FP32))
        dec_rep = ctx.enter_context(nc.sbuf_tensor("dec_rep", [128, 8 * 128], FP32))
        q_tiled = ctx.enter_context(nc.sbuf_tensor("q_tiled", [128, H], FP32))
        scores_buf = ctx.enter_context(nc.sbuf_tensor("scores_buf", [128, NTILES], FP32))
        m_loc = ctx.enter_context(nc.sbuf_tensor("m_loc", [128, 1], FP32))
        m_row = ctx.enter_context(nc.sbuf_tensor("m_row", [1, 128], FP32))
        m_sc = ctx.enter_context(nc.sbuf_tensor("m_sc", [1, 1], FP32))
        m_bc = ctx.enter_context(nc.sbuf_tensor("m_bc", [128, 1], FP32))
        m_all = ctx.enter_context(nc.sbuf_tensor("m_all", [128, NCORES], FP32))
        negM = ctx.enter_context(nc.sbuf_tensor("negM", [128, 1], FP32))
        s3 = ctx.enter_context(nc.sbuf_tensor("s3", [B, SUB * NTILES], FP32))
        s128 = ctx.enter_context(nc.sbuf_tensor("s128", [128, 1], FP32))
        s_all = ctx.enter_context(nc.sbuf_tensor("s_all", [128, NCORES], FP32))
        s_glob = ctx.enter_context(nc.sbuf_tensor("s_glob", [128, 1], FP32))
        rinv16 = ctx.enter_context(nc.sbuf_tensor("rinv16", [B, 1], FP32))
        e3 = ctx.enter_context(nc.sbuf_tensor("e3", [B, SUB * NTILES], FP32))
        o2 = ctx.enter_context(nc.sbuf_tensor("o2", [B, T_L], FP32))
        psum0 = ctx.enter_context(nc.psum_tensor("psum0", [128, 512], FP32))
        psum1 = ctx.enter_context(nc.psum_tensor("psum1", [128, 512], FP32))

        sem_dec = ctx.enter_context(nc.semaphore("sem_dec"))
        wsems = [ctx.enter_context(nc.semaphore(f"sem_w{i}")) for i in range(8)]
        slot_sems = [ctx.enter_context(nc.semaphore(f"sem_s{i}")) for i in range(RING)]
        prep_sem = ctx.enter_context(nc.semaphore("prep_sem"))
        gatherM = ctx.enter_context(nc.semaphore("gatherM"))
        gatherS = ctx.enter_context(nc.semaphore("gatherS"))
        lsem_rdma = ctx.enter_context(nc.semaphore("lsem_rdma"))
        soft_sems = [ctx.enter_context(nc.semaphore(f"sem_soft{i}")) for i in range(SUB)]
        sem_mrow = ctx.enter_context(nc.semaphore("sem_mrow"))
        sem_final = ctx.enter_context(nc.semaphore("sem_final"))
        v_prep = ctx.enter_context(nc.semaphore("v_prep"))
        pe_done = ctx.enter_context(nc.semaphore("pe_done"))
        v_done = ctx.enter_context(nc.semaphore("v_done"))
        v_m1 = ctx.enter_context(nc.semaphore("v_m1"))
        v_m2 = ctx.enter_context(nc.semaphore("v_m2"))
        g_mbc = ctx.enter_context(nc.semaphore("g_mbc"))
        v_m3 = ctx.enter_context(nc.semaphore("v_m3"))
        a_e = ctx.enter_context(nc.semaphore("a_e"))
        v_s = ctx.enter_context(nc.semaphore("v_s"))
        v_fin = ctx.enter_context(nc.semaphore("v_fin"))
        block = ctx.enter_context(nc.Block())

        rings = [ring0, ring1, ring2, w_sb]
        psums = [psum0, psum1]

        @block.sync
        def _(sync):
            # dec first (tiny), then W in 8 pipelined chunks, then enc chunks
            sync.dma_start(
                out=dec_sb[:],
                in_=dec[:].rearrange("(dc p) b -> p dc b", p=128),
            ).then_inc(sem_dec, 16)

            def w_chunk(dc):
                sync.dma_start(
                    out=w_sb[:, dc * H:(dc + 1) * H],
                    in_=w[dc * 128:(dc + 1) * 128, :],
                ).then_inc(wsems[dc], 16)
            def enc_chunk(c):
                s = c % RING
                if s == (NBUF - 1) * PER_BUF and c == s:
                    # first slot living in w_sb: wait for q matmuls to
                    # release W
                    sync.wait_ge(pe_done, 1)
                if c >= RING:
                    sync.wait_ge(v_done, c - RING + 1)
                off = (s % PER_BUF) * CSUB * H
                sync.dma_start(
                    out=rings[s // PER_BUF][:, off:off + CSUB * H],
                    in_=enc[c * CROWS:(c + 1) * CROWS, :].rearrange(
                        "(j p) h -> p j h", p=128
                    ),
                ).then_inc(slot_sems[s], 16)

            for dc in range(8):
                w_chunk(dc)
            for c in range(NCHUNK):
                enc_chunk(c)
            # first half of the scores_buf -> s3 rearrange (rest on scalar)
            sync.wait_ge(v_done, NCHUNK)
            for k in range(SUB // 2):
                sync.dma_start(
                    out=s3[:, k * NTILES:(k + 1) * NTILES],
                    in_=scores_buf[k * B:(k + 1) * B, :],
                ).then_inc(soft_sems[k], 16)

        @block.vector
        def _(vector):
            # dec_rep[p, (dc k b)] = dec_sb[p, (dc b)]  (repeat 8x along k)
            vector.wait_ge(sem_dec, 16)
            vector.tensor_copy(
                dec_rep[:].rearrange("p (dc k b) -> p dc k b", dc=8, k=8),
                dec_sb[:]
                .rearrange("p (dc b) -> p dc b", dc=8)
                .unsqueeze(2)
                .broadcast_to([128, 8, 8, B]),
            ).then_inc(v_prep, 1)

            # q_tiled[p, h] = q[h, p%16] from PSUM
            vector.wait_ge(pe_done, 1)
            vector.tensor_copy(q_tiled[:, 0:512], psum0[:])
            vector.tensor_copy(q_tiled[:, 512:1024], psum1[:])
            vector.drain()

            # main streaming loop: fused multiply + free-axis reduce, in-place
            for c in range(NCHUNK):
                s = c % RING
                vector.wait_ge(slot_sems[s], 16 * (c // RING + 1))
                for j in range(CSUB):
                    i = c * CSUB + j
                    off = (s % PER_BUF) * CSUB * H + j * H
                    ins = vector.scalar_tensor_tensor(
                        out=rings[s // PER_BUF][:, off:off + H],
                        in0=rings[s // PER_BUF][:, off:off + H],
                        scalar=0.0,
                        in1=q_tiled[:],
                        op0=mybir.AluOpType.add,
                        op1=mybir.AluOpType.mult,
                        accum_out=scores_buf[:, i:i + 1],
                    )
                    if j == CSUB - 1:
                        ins.then_inc(v_done, 1)

            # ---- stats-exchange softmax ----
            # local per-partition max over the 64 score columns
            vector.memset(s128[:], 0.0)
            vector.drain()
            vector.tensor_reduce(
                m_loc[:],
                scores_buf[:],
                axis=mybir.AxisListType.X,
                op=mybir.AluOpType.max,
            ).then_inc(v_m1, 1)
            # cross-partition: m_row arrives via SBUF->SBUF DMA; reduce to a
            # scalar, gpsimd broadcasts it back across partitions
            vector.wait_ge(sem_mrow, 16)
            vector.tensor_reduce(
                m_sc[:],
                m_row[:],
                axis=mybir.AxisListType.X,
                op=mybir.AluOpType.max,
            ).then_inc(v_m2, 1)
            # after the remote M exchange: global max -> -M
            vector.wait_ge(gatherM, 16)
            vector.tensor_reduce(
                negM[:],
                m_all[:],
                axis=mybir.AxisListType.X,
                op=mybir.AluOpType.max,
                negate=True,
            ).then_inc(v_m3, 1)
            vector.wait_ge(gatherS, 16)
            vector.tensor_reduce(
                s_glob[:],
                s_all[:],
                axis=mybir.AxisListType.X,
                op=mybir.AluOpType.add,
            )
            vector.drain()
            vector.reciprocal(rinv16[:], s_glob[0:B, :])
            vector.drain()
            # final normalize on the local slice; e3 free layout (k, i),
            # t_l = i*8 + k
            vector.wait_ge(a_e, 1)
            vector.tensor_scalar_mul(
                o2[:].rearrange("b (i k) -> b i k", k=SUB),
                e3[:].rearrange("b (k i) -> b i k", k=SUB),
                rinv16[:],
            ).then_inc(v_fin, 1)

        @block.tensor
        def _(tensor):
            # q_tiled[p, h] = sum_d dec[d, p%16] * W[d, h], chunk-pipelined on W
            tensor.wait_ge(v_prep, 1)
            last = None
            for dc in range(8):
                tensor.wait_ge(wsems[dc], 16)
                for half in range(2):
                    last = tensor.matmul(
                        psums[half][:],
                        dec_rep[:, dc * 128:(dc + 1) * 128],
                        w_sb[:, dc * H + half * 512: dc * H + half * 512 + 512],
                        start=(dc == 0),
                        stop=(dc == 7),
                    )
            last.then_inc(pe_done, 1)

        @block.gpsimd
        def _(gpsimd):
            # pre-generate the broadcast descriptors (hides SWDGE latency);
            # slot = my core id (the AP offset is runtime-computed)
            gpsimd.load_library(library_config.proxy)
            pid = gpsimd.partition_id()
            gpsimd.remote_dma_broadcast(
                out_ap=m_all[:, bass.ds(pid, 1)],
                in_ap=m_bc[:],
                remote_sem=gatherM,
                local_sem=lsem_rdma,
                rdests=[(0, k) for k in range(NCORES)],
            ).then_inc(prep_sem, 1)
            gpsimd.remote_dma_broadcast(
                out_ap=s_all[:, bass.ds(pid, 1)],
                in_ap=s128[:],
                remote_sem=gatherS,
                local_sem=lsem_rdma,
                rdests=[(0, k) for k in range(NCORES)],
            ).then_inc(prep_sem, 1)
            gpsimd.wait_ge(prep_sem, 2)
            # entry barrier (ncfw prelude AllGather, overlaps the stream phase):
            # remote SBUF writes are only safe once every peer started its NEFF
            gpsimd.bir_kernel_barrier_wait([list(range(NCORES))])
            # broadcast the local max scalar across partitions, then exchange
            gpsimd.wait_ge(v_m2, 1)
            gpsimd.partition_broadcast(m_bc[:], m_sc[:]).then_inc(g_mbc, 1)
            gpsimd.wait_ge(g_mbc, 1)
            gpsimd.trigger_dma()
            gpsimd.wait_ge(a_e, 1)
            gpsimd.trigger_dma()
            gpsimd.wait_ge(lsem_rdma, 32)  # sends complete

        @block.scalar
        def _(scalar):
            # local max cross-partition hop: [128,1] -> [1,128]
            scalar.wait_ge(v_m1, 1)
            scalar.dma_start(
                out=m_row[:],
                in_=m_loc[:],
            ).then_inc(sem_mrow, 16)
            # second half of the raw-score rearrange
            scalar.wait_ge(v_done, NCHUNK)
            for k in range(SUB // 2, SUB):
                scalar.dma_start(
                    out=s3[:, k * NTILES:(k + 1) * NTILES],
                    in_=scores_buf[k * B:(k + 1) * B, :],
                ).then_inc(soft_sems[k], 16)
            # exp(x - M) in per-b layout; row sums land directly in s128
            for k in range(SUB):
                scalar.wait_ge(soft_sems[k], 16)
            scalar.wait_ge(v_m3, 1)
            scalar.activation(
                e3[:],
                s3[:],
                mybir.ActivationFunctionType.Exp,
                bias=negM[0:B, :],
                scale=1.0,
                accum_out=s128[0:B, :],
            ).then_inc(a_e, 1)
            # store this core's output slice
            scalar.wait_ge(v_fin, 1)
            scalar.dma_start(out=out[:], in_=o2[:]).then_inc(sem_final, 16)
            scalar.wait_ge(sem_final, 16)

    nc.compile()
    return nc


def make_in_maps(dec_hidden, encoder_outputs, W):
    dec_np = np.ascontiguousarray(np.asarray(dec_hidden, dtype=np.float32))
    enc_np = np.ascontiguousarray(np.asarray(encoder_outputs, dtype=np.float32))
    w_np = np.ascontiguousarray(np.asarray(W, dtype=np.float32))
    assert dec_np.shape == (H, B)
    assert enc_np.shape == (T, B, H)
    assert w_np.shape == (H, H)
    in_maps = []
    for c in range(NCORES):
        shard = np.ascontiguousarray(
            enc_np[c * T_L:(c + 1) * T_L].reshape(ROWS, H)
        )
        in_maps.append({"enc": shard, "dec": dec_np, "w": w_np})
    return in_maps


def _install_ntff_hook():
    """The image's antenv lacks axon_hooks; shim it and register the
    ctypes NTFF profile hook so trace=True works under axon."""
    import types

    if "antenv.axon_hooks" in sys.modules:
        return
    import antenv

    mod = types.ModuleType("antenv.axon_hooks")
    state = {"hook": None}
    mod.set_axon_ntff_profile_hook = lambda h: state.__setitem__("hook", h)
    mod.get_axon_ntff_profile_hook = lambda: state["hook"]
    sys.modules["antenv.axon_hooks"] = mod
    antenv.axon_hooks = mod
    try:
        from trn_agent_boot.trn_boot import _ntff_profile_via_ctypes

        mod.set_axon_ntff_profile_hook(
            _ntff_profile_via_ctypes("/opt/axon/libaxon_pjrt.so")
        )
    except Exception as e:  # degrade to no tracing
        print(f"ntff hook install failed: {e}", file=sys.stderr)


_NC_CACHE = []


def run(dec_hidden, encoder_outputs, W, trace=False):
    if trace:
        _install_ntff_hook()
    if not _NC_CACHE:
        _NC_CACHE.append(build_nc())
    nc = _NC_CACHE[0]
    in_maps = make_in_maps(dec_hidden, encoder_outputs, W)
    res = run_bass_kernel_spmd(
        nc, in_maps, core_ids=list(range(NCORES)), trace=trace
    )
    out = np.concatenate(
        [np.asarray(res.results[c]["out"], dtype=np.float32)
         for c in range(NCORES)],
        axis=1,
    )
    return out, res


def kernel(dec_hidden, encoder_outputs, W):
    out, _ = run(dec_hidden, encoder_outputs, W, trace=False)
    return out
